# revision 19
# baseline (speedup 1.0000x reference)
"""BERT attention layer (B=4, S=1024, H=1024, NH=16) on 8 TRN2 NeuronCores.

Sharding: core c -> batch b = c//2, sequence-half = c%2.  Each core computes
full K/V for its batch (duplicated within the core pair -- cheaper than any
2-rank collective), attention + output projection + residual layernorm for
its 512 query rows, and writes a disjoint [512, 1024] slice of the output.

The host rolls the sequence axis by 512 for odd cores so that every core's
query rows are rows 0..511 of its input -- softmax over keys is permutation
invariant, so one SPMD program serves all 8 cores.

Host-side layout prep (part of sharding): weights and x are pre-transposed
and cast to bf16 so the device kernel's matmul operands are already in
[contraction-on-partition] layout.  All matmuls run bf16 with fp32 PSUM
accumulation; softmax and layernorm run fp32.
"""

import sys

for _p in ("/opt/trn_rl_repo", "/root/.axon_site/_ro/trn_rl_repo"):
    if _p not in sys.path:
        sys.path.insert(0, _p)

import numpy as np

B, S, H, NH, HS = 4, 1024, 1024, 16, 64
P = 128
QR = 512          # query rows per core
EPS = 1e-12
N_CORES = 8

_CACHE = {}


def _build_nc():
    import concourse.mybir as mybir
    import concourse.tile as tile
    from concourse import bacc

    f32 = mybir.dt.float32
    bf16 = mybir.dt.bfloat16
    Alu = mybir.AluOpType
    Act = mybir.ActivationFunctionType

    nc = bacc.Bacc("TRN2", target_bir_lowering=False)

    # ---- per-core DRAM I/O (all layouts host-prepared) ----
    xt_d = nc.declare_dram_parameter("xt", [H, S], bf16, isOutput=False)        # x^T
    xres_d = nc.declare_dram_parameter("xres", [QR, H], f32, isOutput=False)    # q-rows of x
    qwt_d = nc.declare_dram_parameter("qwt", [H, H], bf16, isOutput=False)      # qw^T [i,o]
    kwt_d = nc.declare_dram_parameter("kwt", [H, H], bf16, isOutput=False)
    vwt_d = nc.declare_dram_parameter("vwt", [H, H], bf16, isOutput=False)
    owt_d = nc.declare_dram_parameter("owt", [H, H], bf16, isOutput=False)
    qb_d = nc.declare_dram_parameter("qb", [P, H // P], f32, isOutput=False)    # partition-major
    kb_d = nc.declare_dram_parameter("kb", [P, H // P], f32, isOutput=False)
    mask_d = nc.declare_dram_parameter("maskb", [P, S // P], f32, isOutput=False)
    vbb_d = nc.declare_dram_parameter("vbb", [P, H], f32, isOutput=False)       # broadcast
    obb_d = nc.declare_dram_parameter("obb", [P, H], f32, isOutput=False)
    gammab_d = nc.declare_dram_parameter("gammab", [P, H], f32, isOutput=False)
    betab_d = nc.declare_dram_parameter("betab", [P, H], f32, isOutput=False)
    out_d = nc.declare_dram_parameter("out", [QR, H], f32, isOutput=True)

    KT8 = H // P   # 8 tiles along any 1024 dim

    with tile.TileContext(nc) as tc:
        with (
            tc.tile_pool(name="consts", bufs=1) as consts,
            tc.tile_pool(name="bigs", bufs=1) as bigs,
            tc.tile_pool(name="wbuf", bufs=2) as wbuf,
            tc.tile_pool(name="exps", bufs=2) as exps,
            tc.tile_pool(name="small", bufs=4) as small,
            tc.tile_pool(name="resid", bufs=2) as resid,
            tc.tile_pool(name="onorm", bufs=2) as onorm_pool,
            tc.tile_pool(name="mm_ps", bufs=2, space="PSUM") as mm_ps,
            tc.tile_pool(name="sc_ps", bufs=2, space="PSUM") as sc_ps,
            tc.tile_pool(name="ctx_ps", bufs=1, space="PSUM") as ctx_ps,
            tc.tile_pool(name="dram", bufs=2, space="DRAM") as dram_pool,
        ):
            # ---- constants / small inputs ----
            qb_sb = consts.tile([P, KT8], f32)
            nc.sync.dma_start(out=qb_sb, in_=qb_d[:, :])
            kb_sb = consts.tile([P, KT8], f32)
            nc.sync.dma_start(out=kb_sb, in_=kb_d[:, :])
            mask_sb = consts.tile([P, S // P], f32)
            nc.sync.dma_start(out=mask_sb, in_=mask_d[:, :])
            vbb_sb = consts.tile([P, H], f32)
            nc.sync.dma_start(out=vbb_sb, in_=vbb_d[:, :])
            obb_sb = consts.tile([P, H], f32)
            nc.sync.dma_start(out=obb_sb, in_=obb_d[:, :])
            gammab_sb = consts.tile([P, H], f32)
            nc.sync.dma_start(out=gammab_sb, in_=gammab_d[:, :])
            betab_sb = consts.tile([P, H], f32)
            nc.sync.dma_start(out=betab_sb, in_=betab_d[:, :])
            eps_sb = consts.tile([P, 1], f32)
            nc.vector.memset(eps_sb, EPS)
            # warm the ACT exp table during the load phase so the first real
            # exp doesn't pay the ~2.7us ACT_TABLE_LOAD
            actwarm = consts.tile([P, 1], f32)
            nc.scalar.activation(out=actwarm, in_=eps_sb, func=Act.Exp)

            # ---- x^T in SBUF: [128(i), 8(i_outer), 1024(s)] bf16 ----
            # chunked DMAs so the first matmuls start after the first chunk
            xt_sb = bigs.tile([P, KT8, S], bf16)
            xt_r = xt_d.rearrange("(io p) s -> p io s", p=P)
            for io in range(KT8):
                nc.sync.dma_start(out=xt_sb[:, io, :], in_=xt_r[:, io, :])

            # ---- persistent big tensors ----
            QT = bigs.tile([P, KT8, QR], bf16)        # Q^T  [o, q]
            KT = bigs.tile([P, KT8, S], bf16)         # K^T  [o, s]
            Vaug = bigs.tile([P, KT8, NH, HS + 1], bf16)  # V   [s, h, d(+ones)]
            ctxT = bigs.tile([P, KT8, QR], bf16)      # ctx^T [j, q]
            ysb = bigs.tile([P, QR // P, H], f32)     # pre-LN y rows

            nc.vector.memset(Vaug[:, :, :, HS : HS + 1], 1.0)

            def load_wt(dram):
                w = wbuf.tile([P, KT8, H], bf16, tag="wt")
                w_r = dram.rearrange("(io p) o -> p io o", p=P)
                for io in range(KT8):
                    nc.sync.dma_start(out=w[:, io, :], in_=w_r[:, io, :])
                return w

            # ---- V[s, o] first (ctx needs all of it); natural layout,
            # +vb, written into the augmented buffer ----
            vwt = load_wt(vwt_d)
            for st in range(KT8):
                for oh in range(2):
                    ps = mm_ps.tile([P, QR], f32, tag="mm")
                    for it in range(KT8):
                        nc.tensor.matmul(
                            ps,
                            xt_sb[:, it, st * P : (st + 1) * P],
                            vwt[:, it, oh * QR : (oh + 1) * QR],
                            start=(it == 0),
                            stop=(it == KT8 - 1),
                        )
                    nc.vector.tensor_tensor(
                        out=Vaug[:, st, oh * (NH // 2) : (oh + 1) * (NH // 2), 0:HS],
                        in0=ps.rearrange("p (h d) -> p h d", d=HS),
                        in1=vbb_sb[:, oh * QR : (oh + 1) * QR].rearrange(
                            "p (h d) -> p h d", d=HS
                        ),
                        op=Alu.add,
                    )

            # ---- Q^T[o, q] = sum_i qw^T[i, o-tile] x^T[i, q] (+qb) ----
            qwt = load_wt(qwt_d)
            for ot in range(KT8):
                ps = mm_ps.tile([P, QR], f32, tag="mm")
                for it in range(KT8):
                    nc.tensor.matmul(
                        ps,
                        qwt[:, it, ot * P : (ot + 1) * P],
                        xt_sb[:, it, 0:QR],
                        start=(it == 0),
                        stop=(it == KT8 - 1),
                    )
                nc.vector.tensor_scalar_add(
                    out=QT[:, ot, :], in0=ps, scalar1=qb_sb[:, ot : ot + 1]
                )

            kwt = load_wt(kwt_d)
            # prefetch output weight while attention runs
            owt = load_wt(owt_d)

            # ---- attention: per head pair, K^T projection for just this
            # pair (so the exp stream starts early), scores for both heads
            # packed into one [128, 1024] psum (K=64 matmuls in different PE
            # row groups run concurrently), one merged exp per k-tile, ctx
            # lagging one k-tile so ACT latency hides behind PE work ----
            def emit_kt(ot):
                # K^T[o in pair ot, s]
                for sh in range(2):
                    kps = mm_ps.tile([P, QR], f32, tag="mm", name="kps")
                    for it in range(KT8):
                        nc.tensor.matmul(
                            kps,
                            kwt[:, it, ot * P : (ot + 1) * P],
                            xt_sb[:, it, sh * QR : (sh + 1) * QR],
                            start=(it == 0),
                            stop=(it == KT8 - 1),
                        )
                    nc.vector.tensor_scalar_add(
                        out=KT[:, ot, sh * QR : (sh + 1) * QR],
                        in0=kps,
                        scalar1=kb_sb[:, ot : ot + 1],
                    )

            emit_kt(0)
            for oo in range(NH // 2):
                heads = (2 * oo, 2 * oo + 1)  # partition offsets 0, 64

                expS = exps.tile([P, KT8, 2, QR], bf16, tag="expS", name="expS")
                cpss = [
                    ctx_ps.tile([P, QR], f32, tag=f"ctx{j}", name=f"ctx{j}")
                    for j in range(2)
                ]

                def emit_scores(so):
                    # both heads' scores^T into one [128, 1024] psum tile
                    sps = sc_ps.tile([P, 2 * QR], f32, tag="sc", name="sps")
                    for j in range(2):
                        po = j * HS
                        nc.tensor.matmul(
                            sps[:, j * QR : (j + 1) * QR],
                            KT[po : po + HS, oo, so * P : (so + 1) * P],
                            QT[po : po + HS, oo, :],
                            start=True,
                            stop=True,
                        )
                    # merged exp(scores/8 + mask) for both heads (ScalarE)
                    nc.scalar.activation(
                        out=expS[:, so, :, :],
                        in_=sps.rearrange("p (j q) -> p j q", q=QR),
                        func=Act.Exp,
                        bias=mask_sb[:, so : so + 1],
                        scale=0.125,
                    )

                def emit_ctx(so):
                    for j, h in enumerate(heads):
                        # ctx^T[d(+denom), q] += V_aug[s, d'] . expS^T[s, q]
                        nc.tensor.matmul(
                            cpss[j][0 : HS + 1, :],
                            Vaug[:, so, h, :],
                            expS[:, so, j, :],
                            start=(so == 0),
                            stop=(so == KT8 - 1),
                        )

                emit_scores(0)
                emit_scores(1)
                emit_ctx(0)
                # next pair's K^T here: independent PE work that fills any
                # bubble while this pair's exp stream catches up
                if oo + 1 < NH // 2:
                    emit_kt(oo + 1)
                for so in range(2, KT8):
                    emit_scores(so)
                    emit_ctx(so - 1)
                emit_ctx(KT8 - 1)
                # normalize by softmax denominator (row HS of cps)
                for j, h in enumerate(heads):
                    cps = cpss[j]
                    po = j * HS
                    rd = small.tile([1, QR], f32, tag="rd")
                    nc.vector.reciprocal(out=rd, in_=cps[HS : HS + 1, :])
                    rdd = dram_pool.tile([1, QR], f32, tag="rdd")
                    nc.sync.dma_start(out=rdd, in_=rd)
                    rdb = small.tile([P, QR], f32, tag="rdb")
                    nc.sync.dma_start(
                        out=rdb[0:HS, :], in_=rdd[:, :].to_broadcast([HS, QR])
                    )
                    if po == 0:
                        nc.vector.tensor_tensor(
                            out=ctxT[0:HS, oo, :],
                            in0=cps[0:HS, :],
                            in1=rdb[0:HS, :],
                            op=Alu.mult,
                        )
                    else:
                        stage = small.tile([HS, QR], bf16, tag="cstage")
                        nc.vector.tensor_tensor(
                            out=stage, in0=cps[0:HS, :], in1=rdb[0:HS, :],
                            op=Alu.mult,
                        )
                        nc.sync.dma_start(out=ctxT[po : po + HS, oo, :], in_=stage)

            # ---- y = ctx @ ow^T + ob + residual ----
            xres_r = xres_d.rearrange("(st p) m -> p st m", p=P)
            for st in range(QR // P):
                xr = resid.tile([P, H], f32, tag="xr")
                nc.sync.dma_start(out=xr, in_=xres_r[:, st, :])
                for oh in range(2):
                    ps = mm_ps.tile([P, QR], f32, tag="mm")
                    for jo in range(KT8):
                        nc.tensor.matmul(
                            ps,
                            ctxT[:, jo, st * P : (st + 1) * P],
                            owt[:, jo, oh * QR : (oh + 1) * QR],
                            start=(jo == 0),
                            stop=(jo == KT8 - 1),
                        )
                    nc.vector.tensor_tensor(
                        out=ysb[:, st, oh * QR : (oh + 1) * QR],
                        in0=ps,
                        in1=obb_sb[:, oh * QR : (oh + 1) * QR],
                        op=Alu.add,
                    )
                    nc.vector.tensor_tensor(
                        out=ysb[:, st, oh * QR : (oh + 1) * QR],
                        in0=ysb[:, st, oh * QR : (oh + 1) * QR],
                        in1=xr[:, oh * QR : (oh + 1) * QR],
                        op=Alu.add,
                    )

                # ---- layernorm over the 1024 free dim ----
                BN_F = 512
                ng = H // BN_F
                yr = ysb[:, st, :].rearrange("p (g d) -> p g d", d=BN_F)
                stats = small.tile([P, ng, 6], f32, tag="stats")
                for g in range(ng):
                    nc.vector.bn_stats(out=stats[:, g, :], in_=yr[:, g, :])
                mv = small.tile([P, 2], f32, tag="mv")
                nc.vector.bn_aggr(out=mv, in_=stats)
                rstd = small.tile([P, 1], f32, tag="rstd")
                nc.scalar.activation(
                    out=rstd, in_=mv[:, 1:2], func=Act.Sqrt, bias=eps_sb, scale=1.0
                )
                nc.vector.reciprocal(out=rstd, in_=rstd)
                on = onorm_pool.tile([P, H], f32, tag="on")
                nc.vector.tensor_scalar(
                    out=on,
                    in0=ysb[:, st, :],
                    scalar1=mv[:, 0:1],
                    scalar2=rstd,
                    op0=Alu.subtract,
                    op1=Alu.mult,
                )
                nc.vector.tensor_tensor(out=on, in0=on, in1=gammab_sb, op=Alu.mult)
                nc.vector.tensor_tensor(out=on, in0=on, in1=betab_sb, op=Alu.add)
                nc.sync.dma_start(
                    out=out_d.rearrange("(st p) m -> p st m", p=P)[:, st, :], in_=on
                )

    nc.compile()
    return nc


def _get_nc():
    if "nc" not in _CACHE:
        _CACHE["nc"] = _build_nc()
    return _CACHE["nc"]


def _make_in_maps(inputs):
    import ml_dtypes

    bf16 = ml_dtypes.bfloat16
    hs = np.asarray(inputs["hidden_states"], dtype=np.float32).reshape(B, S, H)
    am = np.asarray(inputs["attention_mask"], dtype=np.float32).reshape(B, S)
    qw = np.asarray(inputs["qw"], dtype=np.float32)
    kw = np.asarray(inputs["kw"], dtype=np.float32)
    vw = np.asarray(inputs["vw"], dtype=np.float32)
    ow = np.asarray(inputs["ow"], dtype=np.float32)
    qb = np.asarray(inputs["qb"], dtype=np.float32)
    kb = np.asarray(inputs["kb"], dtype=np.float32)
    vb = np.asarray(inputs["vb"], dtype=np.float32)
    ob = np.asarray(inputs["ob"], dtype=np.float32)
    gamma = np.asarray(inputs["gamma"], dtype=np.float32)
    beta = np.asarray(inputs["beta"], dtype=np.float32)

    # shared (per-core identical) tensors
    qwt = np.ascontiguousarray(qw.T).astype(bf16)
    kwt = np.ascontiguousarray(kw.T).astype(bf16)
    vwt = np.ascontiguousarray(vw.T).astype(bf16)
    owt = np.ascontiguousarray(ow.T).astype(bf16)
    qb_pm = np.ascontiguousarray(qb.reshape(H // P, P).T)
    kb_pm = np.ascontiguousarray(kb.reshape(H // P, P).T)
    vbb = np.ascontiguousarray(np.broadcast_to(vb, (P, H)))
    obb = np.ascontiguousarray(np.broadcast_to(ob, (P, H)))
    gammab = np.ascontiguousarray(np.broadcast_to(gamma, (P, H)))
    betab = np.ascontiguousarray(np.broadcast_to(beta, (P, H)))

    in_maps = []
    for c in range(N_CORES):
        b, half = divmod(c, 2)
        x = hs[b]
        m = am[b]
        if half:
            x = np.roll(x, -QR, axis=0)
            m = np.roll(m, -QR)
        in_maps.append(
            {
                "xt": np.ascontiguousarray(x.T).astype(bf16),
                "xres": np.ascontiguousarray(x[:QR]),
                "qwt": qwt,
                "kwt": kwt,
                "vwt": vwt,
                "owt": owt,
                "qb": qb_pm,
                "kb": kb_pm,
                "maskb": np.ascontiguousarray(m.reshape(S // P, P).T),
                "vbb": vbb,
                "obb": obb,
                "gammab": gammab,
                "betab": betab,
            }
        )
    return in_maps


def _gather(results):
    out = np.empty((B, S, H), dtype=np.float32)
    for c in range(N_CORES):
        b, half = divmod(c, 2)
        out[b, half * QR : (half + 1) * QR, :] = results[c]["out"]
    return out


def run_on_hw(inputs, **kwargs):
    """Run on the 8 NeuronCores; returns (full_output, BassKernelResults)."""
    from concourse import bass_utils

    nc = _get_nc()
    in_maps = _make_in_maps(inputs)
    res = bass_utils.run_bass_kernel_spmd(
        nc, in_maps, core_ids=list(range(N_CORES)), **kwargs
    )
    return _gather(res.results), res


def kernel(**inputs) -> np.ndarray:
    out, _ = run_on_hw(inputs)
    return out
